# revision 48
# baseline (speedup 1.0000x reference)
"""Trainium2 Bass kernel for nn_MultiHeadSTEVESA.

Data-parallel over batch: 8 elements per core, 8 cores. Within a core,
elements are processed in 4 groups of 2; the slot loop (attention / GRU /
slot-MLP) is batched over the 2 elements (free dim 48).

Key structure:
- Software-pipelined emission: phase A (input MLP + K/V projections) of
  group g+1 is interleaved piece-by-piece with the slot loop of group g,
  with double-buffered K/V SBUF tiles, so the latency-bound slot loop
  hides under phase-A throughput.
- PSUM is organized as lifetime-aligned per-tag rings (ph/px 2KBx2 for the
  MLP stages, pkv 2KBx2 for stats/K/V/logits, lng 2KBx2 for slot-loop
  accumulators) so each stage of chunk n+1 only waits on the same stage of
  chunk n.
- All LayerNorm mean handling is folded into column-centered weights on the
  host (xW' == (x-mean)W). The LN_in scale cancels through the positively
  homogeneous ReLU MLP.
- All rstds are computed without sqrt-family activation tables (single
  exp/tanh ACT table set for the whole kernel, ~1 table load total):
  phase-A rstd uses a one-step bit-trick seed (error centered via a
  constant folded into the variance scale); slot-loop rstds add one
  Newton iteration (output head is scale-sensitive).
- Attention: block-sparse head structure exploited - heads {0,1}/{2,3}
  contract only k-chunk 0/1, so logits are two independent half-width
  matmuls per token tile. Joint softmax over (head,slot) via one Exp +
  free-dim reduce; per-token normalize in-place; update matmul carries a
  fused ones-column for the renormalization denominator.
- GRU sigmoid expressed through tanh; GEMM dtypes bf16 (f32r for the
  phase-A MLP and output head).
"""

import sys

import numpy as np

sys.path.insert(0, "/opt/trn_rl_repo")

import ml_dtypes

import concourse.bass as bass
import concourse.mybir as mybir
import concourse.tile as tile
from concourse import bacc, bass_utils
from concourse.alu_op_type import AluOpType
from concourse.masks import make_identity

AF = mybir.ActivationFunctionType
AX = mybir.AxisListType
f32 = mybir.dt.float32
f32r = mybir.dt.float32r
bf16 = mybir.dt.bfloat16
i32 = mybir.dt.int32
ts = bass.ts
BF = ml_dtypes.bfloat16

# Problem shapes
B, C, RES = 64, 256, 64
S, SLOT, H, MLP_H, OUT = 24, 256, 4, 1024, 256
ITERS = 3
LN_EPS = 1e-5
DH = SLOT // H

P = 128
KC = C // P            # 2 feature chunks
N = RES * RES          # 4096 tokens
NCH = 512              # token chunk for phase A
NB = N // NCH          # 8
NL = N // P            # 32 token chunks for attention
HSP = 128              # padded (head, slot) dim: hs' = h*32 + s
MC_MLP = MLP_H // P    # 8
VW = 257               # vT tile width: 256 v-cols + 1 ones col
NCORES = 8
BP = B // NCORES       # 8 batch elems per core
NB_E = 2               # elements per resident group (slot loop batch)
NG = BP // NB_E        # 2 groups
SB = NB_E * S          # 96: batched slot free dim

# negative-seed rsqrt magic for half-scaled input (v05 = var/2), sign bit set
RSQRT_M = -1619374625  # int32 view of 0xDEF759DF


def _build_program(bp=BP):
    nc = bacc.Bacc(
        "TRN2",
        target_bir_lowering=False,
        debug=False,
        enable_asserts=False,
        num_devices=NCORES,
    )

    d = {}

    def din(name, shape, dt=f32):
        d[name] = nc.dram_tensor(name, shape, dt, kind="ExternalInput").ap()
        return d[name]

    xin = din("xin", [bp, KC, P, N], f32r)
    din("w1t", [P, KC, C], f32r)
    din("c1c", [P, KC])
    din("w2t", [P, KC, C], f32r)
    din("b2c", [P, KC])
    din("wkt", [P, KC, C], bf16)
    din("ckc", [P, KC])
    din("wvt", [P, KC, C], bf16)
    din("cvc", [P, KC])
    din("wqt", [P, KC, C], bf16)
    din("cqc", [P, KC])
    din("wit", [P, KC, 3 * SLOT], bf16)
    din("wht", [P, KC, 3 * SLOT], bf16)
    din("brzh", [P, 4])
    din("bhn", [P, KC])
    din("bin", [P, KC])
    din("m1t", [P, KC, MLP_H], bf16)
    din("c1m", [P, MC_MLP])
    din("m2t", [P, MC_MLP, C], bf16)
    din("b2m", [P, KC])
    din("wot", [P, KC, OUT], f32r)
    din("co", [1, OUT], f32r)
    din("smu", [P, KC, S])

    out_d = nc.dram_tensor("out", [bp, S, OUT], f32, kind="ExternalOutput").ap()

    from contextlib import ExitStack

    with tile.TileContext(nc) as tc, ExitStack() as ctx:
        wp = ctx.enter_context(tc.tile_pool(name="wp", bufs=1))
        kv = ctx.enter_context(tc.tile_pool(name="kv", bufs=2))
        ch = ctx.enter_context(tc.tile_pool(name="ch", bufs=2))
        cw = ctx.enter_context(tc.tile_pool(name="cw", bufs=2))
        xp = ctx.enter_context(tc.tile_pool(name="xp", bufs=2))
        t5 = ctx.enter_context(tc.tile_pool(name="t5", bufs=2))
        att = ctx.enter_context(tc.tile_pool(name="att", bufs=2))
        slo = ctx.enter_context(tc.tile_pool(name="slo", bufs=1))
        slp = ctx.enter_context(tc.tile_pool(name="slp", bufs=1))
        sl2 = ctx.enter_context(tc.tile_pool(name="sl2", bufs=2))
        ps = ctx.enter_context(tc.tile_pool(name="ps", bufs=2, space="PSUM"))

        def ptag(tag, shape, dt=f32):
            return ps.tile(shape, dt, tag=tag, bufs=2, name=tag)

        # ---- persistent constants / weights ----
        ident = wp.tile([P, P], f32, tag="ident")
        make_identity(nc, ident[:])
        ones_r = wp.tile([P, P], f32r, tag="ones_r")
        nc.vector.tensor_scalar(
            ones_r[:], ident[:], 0.0, 1.0, AluOpType.mult, AluOpType.add
        )
        ones_sb = wp.tile([1, SB], f32r, tag="ones_sb")
        nc.vector.tensor_scalar(
            ones_sb[:], ident[0:1, 0:SB], 0.0, 1.0, AluOpType.mult, AluOpType.add
        )
        ones_bf = wp.tile([P, P], bf16, tag="ones_bf")
        nc.vector.tensor_scalar(
            ones_bf[:], ident[:], 0.0, 1.0, AluOpType.mult, AluOpType.add
        )

        W = {}
        for name, ap in d.items():
            if name == "xin":
                continue
            t = wp.tile(list(ap.shape), ap.dtype, tag=name)
            nc.sync.dma_start(t[:], ap)
            W[name] = t

        # rstd = rsqrt(2*v05) via negative bit-trick seed + 1 Newton step.
        # No activation table needed (keeps ACT on the exp/tanh set all
        # kernel long). Destroys v05; sdt is scratch (holds -y0).
        def rsqrt2(dst, v05):
            # seed-only: dst = +bitcast(MAGICPOS - (bits(v05) >> 1)), ~3.4% err
            nc.vector.tensor_scalar(
                dst[:].bitcast(i32), v05[:].bitcast(i32), 1, None,
                AluOpType.logical_shift_right,
            )
            nc.vector.tensor_scalar(
                dst[:].bitcast(i32), dst[:].bitcast(i32), -1, RSQRT_M_POS,
                AluOpType.mult, AluOpType.add,
            )

        def rsqrt5(dst, v05, sdt, meng):
            nc.vector.tensor_scalar(
                sdt[:].bitcast(i32), v05[:].bitcast(i32), 1, None,
                AluOpType.logical_shift_right,
            )
            nc.vector.tensor_scalar(
                sdt[:].bitcast(i32), sdt[:].bitcast(i32), -1, RSQRT_M,
                AluOpType.mult, AluOpType.add,
            )
            meng.tensor_tensor(v05[:], v05[:], sdt[:], AluOpType.mult)
            meng.tensor_tensor(v05[:], v05[:], sdt[:], AluOpType.mult)
            nc.vector.scalar_tensor_tensor(
                dst[:], v05[:], 1.5, sdt[:], AluOpType.subtract, AluOpType.mult
            )

        # ---- phase A chunk emitter (one element, one 512-token chunk) ----
        def emit_chunk(e, nb, kb, vt):
            sl = ts(nb, NCH)
            x0 = ch.tile([P, KC, NCH], f32r, tag="x0c", name="x0")
            for kc in range(KC):
                nc.sync.dma_start(x0[:, kc], xin[e, kc, :, sl])
            # W1 (centered; LN_in scale cancels through the MLP)
            ph = [ptag("ph", [P, NCH]) for _ in range(KC)]
            for mc in range(KC):
                for kc in range(KC):
                    nc.tensor.matmul(
                        ph[mc][:], W["w1t"][:, kc, ts(mc, P)], x0[:, kc],
                        start=(kc == 0), stop=(kc == KC - 1),
                    )
            h = cw.tile([P, KC, NCH], f32r, tag="hc", name="h")
            for mc in range(KC):
                nc.scalar.activation(
                    h[:, mc], ph[mc][:], AF.Relu, bias=W["c1c"][:, mc : mc + 1]
                )
            # W2
            px2 = [ptag("px", [P, NCH]) for _ in range(KC)]
            for mc in range(KC):
                for kc in range(KC):
                    nc.tensor.matmul(
                        px2[mc][:], W["w2t"][:, kc, ts(mc, P)], h[:, kc],
                        start=(kc == 0), stop=(kc == KC - 1),
                    )
            x2 = cw.tile([P, KC, NCH], bf16, tag="x2c", name="x2")
            sq2 = cw.tile([P, KC, NCH], bf16, tag="sq2", name="sq2")
            for mc in range(KC):
                nc.scalar.activation(
                    x2[:, mc], px2[mc][:], AF.Identity, bias=W["b2c"][:, mc : mc + 1]
                )
                nc.scalar.activation(
                    sq2[:, mc], px2[mc][:], AF.Square, bias=W["b2c"][:, mc : mc + 1]
                )
            # LN_inp stats: mean and E[x^2] via ones-matmul
            p1 = ptag("pkv", [P, NCH])
            p2 = ptag("pkv", [P, NCH])
            for kc in range(KC):
                nc.tensor.matmul(
                    p1[:], ones_bf[:], x2[:, kc],
                    start=(kc == 0), stop=(kc == KC - 1),
                )
            for kc in range(KC):
                nc.tensor.matmul(
                    p2[:], ones_bf[:], sq2[:, kc],
                    start=(kc == 0), stop=(kc == KC - 1),
                )
            CSEED = (1.0 + 0.017) ** 2
            sqm = t5.tile([P, NCH], f32, tag="sqm", name="sqm")
            nc.scalar.activation(
                sqm[:], p1[:], AF.Square, scale=(0.5 * CSEED) ** 0.5 / C
            )
            v05 = t5.tile([P, NCH], f32, tag="v05", bufs=1, name="v05")
            nc.vector.scalar_tensor_tensor(
                v05[:], p2[:], 0.5 * CSEED / C, sqm[:],
                AluOpType.mult, AluOpType.subtract,
            )
            ivb = t5.tile([P, NCH], f32, tag="sqm", name="ivb")
            rsqrt2(ivb, v05)
            xh2 = xp.tile([P, KC, NCH], bf16, tag="xh2", name="xh2")
            nc.vector.tensor_tensor(
                xh2[:], x2[:], ivb[:, None, :].broadcast_to([P, KC, NCH]),
                AluOpType.mult,
            )
            # K projection: slot-major output [d, tok]
            pk = [ptag("pkv", [P, NCH]) for _ in range(KC)]
            for mc in range(KC):
                for kc in range(KC):
                    nc.tensor.matmul(
                        pk[mc][:], W["wkt"][:, kc, ts(mc, P)], xh2[:, kc],
                        start=(kc == 0), stop=(kc == KC - 1),
                    )
            nc.scalar.activation(
                kb[:, 0, sl], pk[0][:], AF.Identity, bias=W["ckc"][:, 0:1]
            )
            nc.vector.tensor_scalar_add(kb[:, 1, sl], pk[1][:], W["ckc"][:, 1:2])
            # V projection: token-major output [tok, d] (x stationary)
            for jp in range(2):
                pv = ptag("pkv", [P, 2, C])
                for jj in range(2):
                    j = 2 * jp + jj
                    for kc in range(KC):
                        nc.tensor.matmul(
                            pv[:, jj], xh2[:, kc, ts(j, P)], W["wvt"][:, kc, :],
                            start=(kc == 0), stop=(kc == KC - 1),
                        )
                nc.vector.tensor_copy(
                    vt[:, nb * 4 + 2 * jp : nb * 4 + 2 * jp + 2, 0:256], pv[:]
                )

        def slot_stats(src_f32, tag):
            """src [P, KC, SB] -> iv [P, SB] (rstd via DVE-only rsqrt)."""
            sqs = slo.tile([P, KC, SB], f32r, tag="ssq", name="sqs")
            nc.gpsimd.tensor_mul(sqs[:], src_f32[:], src_f32[:])
            pq1 = ptag("lng", [P, SB])
            for kc in range(KC):
                nc.tensor.matmul(
                    pq1[:], ones_r[:], src_f32[:, kc],
                    start=(kc == 0), stop=(kc == KC - 1),
                )
            pq2 = ptag("lng", [P, SB])
            for kc in range(KC):
                nc.tensor.matmul(
                    pq2[:], ones_r[:], sqs[:, kc],
                    start=(kc == 0), stop=(kc == KC - 1),
                )
            sqm = slo.tile([P, SB], f32, tag="ssqm", name="sqm2")
            nc.scalar.activation(sqm[:], pq1[:], AF.Square, scale=0.5 ** 0.5 / C)
            vv = slo.tile([P, SB], f32, tag="sv", name="vv")
            nc.vector.scalar_tensor_tensor(
                vv[:], pq2[:], 0.5 / C, sqm[:], AluOpType.mult, AluOpType.subtract
            )
            sdt = slo.tile([P, SB], f32, tag="rssd", name="sdt2")
            iv = slo.tile([P, SB], f32, tag=tag + "iv", name="iv")
            rsqrt5(iv, vv, sdt, nc.vector)
            return iv

        # ---- slot-loop piece list for one group (emitted lazily) ----
        def make_slot_pieces(g, kbf, vtt):
            st = {}

            def p_init():
                slots = sl2.tile([P, KC, SB], f32r, tag="slots", name="slots")
                for e4 in range(NB_E):
                    nc.vector.tensor_copy(
                        slots[:, :, e4 * S : (e4 + 1) * S], W["smu"][:]
                    )
                st["slots"] = slots
                qb = []
                for e4 in range(NB_E):
                    q = slp.tile([P, KC, HSP], bf16, tag=f"qb{e4}", name="q")
                    nc.vector.memset(q[:], 0.0)
                    qb.append(q)
                st["qb"] = qb

            def p_stats(it):
                slots, qb = st["slots"], st["qb"]
                ivq = slot_stats(slots, "qs")
                sh = slo.tile([P, KC, SB], bf16, tag="sh", name="sh")
                for kc in range(KC):
                    nc.gpsimd.tensor_mul(sh[:, kc], slots[:, kc], ivq[:])
                qsb = slo.tile([P, KC, SB], bf16, tag="qsb", name="qsb")
                for mc in range(KC):
                    pq = ptag("lng", [P, SB])
                    for kc in range(KC):
                        nc.tensor.matmul(
                            pq[:], W["wqt"][:, kc, ts(mc, P)], sh[:, kc],
                            start=(kc == 0), stop=(kc == KC - 1),
                        )
                    nc.scalar.activation(
                        qsb[:, mc], pq[:], AF.Identity, bias=W["cqc"][:, mc : mc + 1]
                    )
                for e4 in range(NB_E):
                    for hh in range(H):
                        pr = slice((hh % 2) * 64, (hh % 2) * 64 + 64)
                        nc.gpsimd.tensor_copy(
                            qb[e4][pr, hh // 2, hh * 32 : hh * 32 + S],
                            qsb[pr, hh // 2, e4 * S : e4 * S + S],
                        )
                st["updt"] = slp.tile([P, KC, SB], bf16, tag="updt", name="updt")

            def p_att(it, e4):
                qb = st["qb"]
                updt = st["updt"]
                psu = ptag("lng", [P, SLOT + 1])
                for gi in range(8):
                    psl = ptag("pkv", [P, 4, HSP])
                    for j4 in range(4):
                        nl = gi * 4 + j4
                        # heads {0,1} live in k-chunk 0 / hs 0:64; {2,3} in
                        # chunk 1 / hs 64:128 (qb is block-sparse) — two
                        # independent half-width matmuls, no accumulation.
                        for kc in range(KC):
                            nc.tensor.matmul(
                                psl[:, j4, kc * 64 : kc * 64 + 64],
                                kbf[e4][:, kc, ts(nl, P)],
                                qb[e4][:, kc, kc * 64 : kc * 64 + 64],
                                start=True, stop=True,
                            )
                    esb = att.tile([P, 4, HSP], bf16, tag="esb", bufs=5, name="esb")
                    nc.scalar.activation(esb[:], psl[:], AF.Exp)
                    t4 = att.tile([P, 4], f32, tag="t4", bufs=4, name="t4")
                    nc.vector.reduce_sum(t4[:], esb[:], axis=AX.X)
                    t4m = att.tile([P, 4], f32, tag="t4m", bufs=4, name="t4m")
                    nc.vector.tensor_scalar(
                        t4m[:], t4[:], -32.0, None, AluOpType.add
                    )
                    rt4 = att.tile([P, 4], f32, tag="rt4", bufs=4, name="rt4")
                    nc.vector.reciprocal_approx_fast(rt4[:], t4m[:])
                    nc.vector.tensor_tensor(
                        esb[:], esb[:],
                        rt4[:, :, None].broadcast_to([P, 4, HSP]),
                        AluOpType.mult,
                    )
                    for j4 in range(4):
                        nc.tensor.matmul(
                            psu[:], esb[:, j4], vtt[e4][:, gi * 4 + j4, :],
                            start=(gi == 0 and j4 == 0),
                            stop=(gi == 7 and j4 == 3),
                            skip_group_check=True,
                        )
                rz = att.tile([P, 1], f32, tag="rz", name="rz")
                nc.vector.reciprocal_approx_fast(rz[:], psu[:, 256:257])
                upd_s = att.tile([P, SLOT], f32, tag="upd_s", name="upd_s")
                nc.vector.tensor_scalar_mul(upd_s[:], psu[:, 0:SLOT], rz[:])
                for hh in range(H):
                    bp0 = hh * 32
                    pt = ptag("lng", [64, S])
                    nc.tensor.transpose(
                        pt[:],
                        upd_s[bp0 : bp0 + S, ts(hh, DH)],
                        ident[bp0 : bp0 + S, bp0 : bp0 + S],
                        tile_position=(bp0, 0),
                    )
                    nc.scalar.activation(
                        updt[(hh % 2) * 64 : (hh % 2) * 64 + 64, hh // 2,
                             e4 * S : e4 * S + S],
                        pt[:],
                        AF.Identity,
                        bias=W["cvc"][(hh % 2) * 64 : (hh % 2) * 64 + 64,
                                      hh // 2 : hh // 2 + 1],
                    )

            def p_gru(it):
                slots, updt = st["slots"], st["updt"]
                sl16 = slo.tile([P, KC, SB], bf16, tag="sl16", name="sl16")
                nc.gpsimd.tensor_copy(sl16[:], slots[:])
                g12 = ptag("lng", [P, 8, SB])
                ph_rz = g12[:, 0:4]
                px_rz = g12[:, 4:8]
                pn = ptag("lng", [P, 4, SB])
                for gj in range(4):
                    for kc in range(KC):
                        nc.tensor.matmul(
                            ph_rz[:, gj], W["wht"][:, kc, ts(gj, P)], sl16[:, kc],
                            start=(kc == 0), stop=(kc == KC - 1),
                        )
                for gj in range(4):
                    for kc in range(KC):
                        nc.tensor.matmul(
                            px_rz[:, gj], W["wit"][:, kc, ts(gj, P)], updt[:, kc],
                            start=(kc == 0), stop=(kc == KC - 1),
                        )
                for nj in range(KC):
                    for kc in range(KC):
                        nc.tensor.matmul(
                            pn[:, nj], W["wit"][:, kc, ts(4 + nj, P)], updt[:, kc],
                            start=(kc == 0), stop=(kc == KC - 1),
                        )
                    for kc in range(KC):
                        nc.tensor.matmul(
                            pn[:, 2 + nj], W["wht"][:, kc, ts(4 + nj, P)],
                            sl16[:, kc],
                            start=(kc == 0), stop=(kc == KC - 1),
                        )
                hgs = slo.tile([P, 4, SB], bf16, tag="trz", name="hgs")
                nc.scalar.activation(hgs[:], ph_rz, AF.Identity)
                tg = slo.tile([P, 4, SB], f32, tag="tg", name="tg")
                nc.vector.tensor_add(tg[:], px_rz, hgs[:])
                trz = slo.tile([P, 4, SB], f32, tag="trz", name="trz")
                for gj in range(4):
                    nc.scalar.activation(
                        trz[:, gj], tg[:, gj], AF.Tanh, scale=0.5,
                        bias=W["brzh"][:, gj : gj + 1],
                    )
                # n = tanh(0.5*(y + tr*y) + xn + bin), y = hn + bhn
                pns = slo.tile([P, 4, SB], f32, tag="tg", name="pns")
                nc.scalar.activation(pns[:], pn[:], AF.Identity)
                yn = slo.tile([P, KC, SB], f32, tag="yn", name="yn")
                for nj in range(KC):
                    nc.vector.tensor_scalar_add(
                        yn[:, nj], pns[:, 2 + nj], W["bhn"][:, nj : nj + 1]
                    )
                gn = slo.tile([P, KC, SB], f32, tag="gn", name="gn")
                nc.vector.tensor_mul(gn[:], trz[:, 0:2], yn[:])
                nc.vector.tensor_add(gn[:], gn[:], yn[:])
                mn = slo.tile([P, KC, SB], f32, tag="ssq", name="mn")
                for nj in range(KC):
                    nc.vector.scalar_tensor_tensor(
                        mn[:, nj], gn[:, nj], 0.5, pns[:, nj],
                        AluOpType.mult, AluOpType.add,
                    )
                nsb = slo.tile([P, KC, SB], f32, tag="nsb", name="nsb")
                for nj in range(KC):
                    nc.scalar.activation(
                        nsb[:, nj], mn[:, nj], AF.Tanh, bias=W["bin"][:, nj : nj + 1]
                    )
                # slots2 = n + (0.5 + 0.5*tz)*(slots - n)
                dd = slo.tile([P, KC, SB], f32, tag="dd", name="dd")
                nc.vector.tensor_sub(dd[:], slots[:], nsb[:])
                ee = slo.tile([P, KC, SB], f32, tag="ee", name="ee")
                nc.vector.tensor_mul(ee[:], trz[:, 2:4], dd[:])
                nc.vector.tensor_add(ee[:], ee[:], dd[:])
                slots2 = slp.tile([P, KC, SB], f32r, tag="slots2", name="slots2")
                nc.vector.scalar_tensor_tensor(
                    slots2[:], ee[:], 0.5, nsb[:], AluOpType.mult, AluOpType.add
                )
                st["slots2"] = slots2

            def p_mlp(it):
                slots2 = st["slots2"]
                ivm = slot_stats(slots2, "ms")
                sh2 = slo.tile([P, KC, SB], bf16, tag="sh2", name="sh2")
                for kc in range(KC):
                    nc.gpsimd.tensor_mul(sh2[:, kc], slots2[:, kc], ivm[:])
                hm = slo.tile([P, MC_MLP, SB], bf16, tag="hm", name="hm")
                for j in range(MC_MLP):
                    pz = ptag("lng", [P, SB])
                    for kc in range(KC):
                        nc.tensor.matmul(
                            pz[:], W["m1t"][:, kc, ts(j, P)], sh2[:, kc],
                            start=(kc == 0), stop=(kc == KC - 1),
                        )
                    if j % 2 == 0:
                        nc.vector.tensor_scalar(
                            hm[:, j], pz[:], W["c1m"][:, j : j + 1], 0.0,
                            AluOpType.add, AluOpType.max,
                        )
                    else:
                        nc.scalar.activation(
                            hm[:, j], pz[:], AF.Relu, bias=W["c1m"][:, j : j + 1]
                        )
                slots3 = sl2.tile([P, KC, SB], f32r, tag="slots", name="slots3")
                for mc in range(KC):
                    p2z = ptag("lng", [P, SB])
                    for j in range(MC_MLP):
                        nc.tensor.matmul(
                            p2z[:], W["m2t"][:, j, ts(mc, P)], hm[:, j],
                            start=(j == 0), stop=(j == MC_MLP - 1),
                        )
                    tr = slo.tile([P, SB], f32, tag="tr", name="tr")
                    nc.vector.tensor_scalar_add(
                        tr[:], p2z[:], W["b2m"][:, mc : mc + 1]
                    )
                    nc.vector.tensor_add(slots3[:, mc], tr[:], slots2[:, mc])
                st["slots"] = slots3

            def p_head():
                slots = st["slots"]
                ivh = slot_stats(slots, "hs")
                sh3 = slo.tile([P, KC, SB], f32r, tag="sh3", name="sh3")
                for kc in range(KC):
                    nc.gpsimd.tensor_mul(sh3[:, kc], slots[:, kc], ivh[:])
                po = ptag("lng", [SB, OUT])
                for kc in range(KC):
                    nc.tensor.matmul(
                        po[:], sh3[:, kc], W["wot"][:, kc, :],
                        start=(kc == 0), stop=False,
                    )
                nc.tensor.matmul(
                    po[:], ones_sb[:], W["co"][:], start=False, stop=True
                )
                osb = slo.tile([SB, OUT], f32, tag="osb", name="osb")
                nc.scalar.activation(osb[:], po[:], AF.Copy)
                for e4 in range(NB_E):
                    nc.sync.dma_start(
                        out_d[g * NB_E + e4], osb[e4 * S : (e4 + 1) * S, :]
                    )

            pieces = [p_init]
            for it in range(ITERS):
                pieces.append(lambda it=it: p_stats(it))
                for e4 in range(NB_E):
                    pieces.append(lambda it=it, e4=e4: p_att(it, e4))
                pieces.append(lambda it=it: p_gru(it))
                pieces.append(lambda it=it: p_mlp(it))
            pieces.append(p_head)
            return pieces

        # ======== pipelined emission: phase A(g) overlaps slot loop(g-1) =====
        pend = []
        for g in range(bp // NB_E):
            kbf = []
            vtt = []
            for e4 in range(NB_E):
                kb = kv.tile([P, KC, N], bf16, tag=f"kbf{e4}", name="kb")
                vt = kv.tile([P, NL, VW], bf16, tag=f"vtt{e4}", name="vt")
                nc.vector.memset(vt[:, :, 256:257], 1.0)
                kbf.append(kb)
                vtt.append(vt)
            chunks = [(nb, e4) for nb in range(NB) for e4 in range(NB_E)]
            npcs = len(pend)
            pi = 0
            for i, (nb, e4) in enumerate(chunks):
                emit_chunk(g * NB_E + e4, nb, kbf[e4], vtt[e4])
                while pi < npcs and (pi - 1) * len(chunks) < (i + 1) * npcs:
                    pend[pi]()
                    pi += 1
            while pi < npcs:
                pend[pi]()
                pi += 1
            pend = make_slot_pieces(g, kbf, vtt)
        for p in pend:
            p()

    nc.compile()
    return nc


def _center(a):
    """Column-center: x @ center(A) == (x - mean(x)) @ A."""
    return (a - a.mean(0, keepdims=True)).astype(np.float32)


def _host_prepack(i):
    g = lambda k: np.asarray(i[k], np.float32)
    coords = (np.arange(RES, dtype=np.float32) + 0.5) / RES
    gx = np.broadcast_to(coords[None, :], (RES, RES))
    gy = np.broadcast_to(coords[:, None], (RES, RES))
    pe = np.stack([gx, gy, 1.0 - gx, 1.0 - gy], 0).astype(np.float32)
    pos = np.einsum("co,chw->ohw", g("pos_w"), pe).astype(np.float32)
    pos = pos + g("pos_b")[:, None, None]
    x = g("inputs") + pos[None]
    xin = np.ascontiguousarray(x.reshape(B, KC, P, N))

    def kmaj(w, dt=np.float32):
        K, M = w.shape
        return np.ascontiguousarray(
            w.reshape(K // P, P, M).transpose(1, 0, 2).astype(dt)
        )

    def cols(v):
        M = v.shape[0]
        return np.ascontiguousarray(v.reshape(M // P, P).T.astype(np.float32))

    sh = {}
    # W1: gamma-fold + center (mean of LN_in via centering; scale cancels)
    sh["w1t"] = kmaj(_center(g("ln_in_g")[:, None] * g("mlp_in_w1")))
    sh["c1c"] = cols(g("ln_in_b") @ g("mlp_in_w1") + g("mlp_in_b1"))
    sh["w2t"] = kmaj(g("mlp_in_w2"))
    sh["b2c"] = cols(g("mlp_in_b2"))
    ks = float(SLOT) ** -0.5
    sh["wkt"] = kmaj(_center(g("ln_inp_g")[:, None] * g("Wk")) * ks, BF)
    sh["ckc"] = cols((g("ln_inp_b") @ g("Wk")) * ks)
    sh["wvt"] = kmaj(_center(g("ln_inp_g")[:, None] * g("Wv")), BF)
    sh["cvc"] = cols(g("ln_inp_b") @ g("Wv"))
    sh["wqt"] = kmaj(_center(g("ln_slot_g")[:, None] * g("Wq")), BF)
    sh["cqc"] = cols(g("ln_slot_b") @ g("Wq"))
    sh["wit"] = kmaj(g("gru_wi"), BF)
    sh["wht"] = kmaj(g("gru_wh"), BF)
    bsum = g("gru_bi") + g("gru_bh")
    sh["brzh"] = cols(0.5 * bsum[0 : 2 * SLOT])
    sh["bhn"] = cols(g("gru_bh")[2 * SLOT :])
    sh["bin"] = cols(g("gru_bi")[2 * SLOT :])
    sh["m1t"] = kmaj(_center(g("ln_mlp_g")[:, None] * g("mlp_w1")), BF)
    sh["c1m"] = cols(g("ln_mlp_b") @ g("mlp_w1") + g("mlp_b1"))
    sh["m2t"] = kmaj(g("mlp_w2"), BF)
    sh["b2m"] = cols(g("mlp_b2"))
    sh["wot"] = kmaj(_center(g("ln_out_g")[:, None] * g("out_w")))
    sh["co"] = (g("ln_out_b") @ g("out_w") + g("out_b")).reshape(1, OUT)
    mu = np.asarray(i["slot_mu"], np.float32)[0]
    sh["smu"] = np.ascontiguousarray(mu.T.reshape(KC, P, S).transpose(1, 0, 2))
    return sh, xin


_NC_CACHE = {}
LAST_RESULTS = None


def _get_nc():
    if "nc" not in _NC_CACHE:
        _NC_CACHE["nc"] = _build_program(BP)
    return _NC_CACHE["nc"]


def kernel(**inputs):
    global LAST_RESULTS
    nc = _get_nc()
    sh, xin = _host_prepack(inputs)
    in_maps = []
    for c in range(NCORES):
        m = dict(sh)
        m["xin"] = np.ascontiguousarray(xin[c * BP : (c + 1) * BP])
        in_maps.append(m)
    res = bass_utils.run_bass_kernel_spmd(
        nc, in_maps, core_ids=list(range(NCORES))
    )
    LAST_RESULTS = res
    out = np.concatenate([res.results[c]["out"] for c in range(NCORES)], 0)
    return out.astype(np.float32)



# revision 49
# speedup vs baseline: 2.3684x; 2.3684x over previous
"""Trainium2 Bass kernel for nn_MultiHeadSTEVESA.

Data-parallel over batch: 8 elements per core, 8 cores. Within a core,
elements are processed in 4 groups of 2; the slot loop (attention / GRU /
slot-MLP) is batched over the 2 elements (free dim 48).

Key structure:
- Software-pipelined emission: phase A (input MLP + K/V projections) of
  group g+1 is interleaved piece-by-piece with the slot loop of group g,
  with double-buffered K/V SBUF tiles, so the latency-bound slot loop
  hides under phase-A throughput.
- PSUM is organized as lifetime-aligned per-tag rings (ph/px 2KBx2 for the
  MLP stages, pkv 2KBx2 for stats/K/V/logits, lng 2KBx2 for slot-loop
  accumulators) so each stage of chunk n+1 only waits on the same stage of
  chunk n.
- All LayerNorm mean handling is folded into column-centered weights on the
  host (xW' == (x-mean)W). The LN_in scale cancels through the positively
  homogeneous ReLU MLP.
- All rstds are computed without sqrt-family activation tables (single
  exp/tanh ACT table set for the whole kernel, ~1 table load total):
  phase-A rstd uses a one-step bit-trick seed (error centered via a
  constant folded into the variance scale); slot-loop rstds add one
  Newton iteration (output head is scale-sensitive).
- Attention: block-sparse head structure exploited - heads {0,1}/{2,3}
  contract only k-chunk 0/1, so logits are two independent half-width
  matmuls per token tile. Joint softmax over (head,slot) via one Exp +
  free-dim reduce; per-token normalize in-place; update matmul carries a
  fused ones-column for the renormalization denominator.
- GRU sigmoid expressed through tanh; GEMM dtypes bf16 (f32r for the
  phase-A MLP and output head).
"""

import sys

import numpy as np

sys.path.insert(0, "/opt/trn_rl_repo")

import ml_dtypes

import concourse.bass as bass
import concourse.mybir as mybir
import concourse.tile as tile
from concourse import bacc, bass_utils
from concourse.alu_op_type import AluOpType
from concourse.masks import make_identity

AF = mybir.ActivationFunctionType
AX = mybir.AxisListType
f32 = mybir.dt.float32
f32r = mybir.dt.float32r
bf16 = mybir.dt.bfloat16
i32 = mybir.dt.int32
ts = bass.ts
BF = ml_dtypes.bfloat16

# Problem shapes
B, C, RES = 64, 256, 64
S, SLOT, H, MLP_H, OUT = 24, 256, 4, 1024, 256
ITERS = 3
LN_EPS = 1e-5
DH = SLOT // H

P = 128
KC = C // P            # 2 feature chunks
N = RES * RES          # 4096 tokens
NCH = 512              # token chunk for phase A
NB = N // NCH          # 8
NL = N // P            # 32 token chunks for attention
HSP = 128              # padded (head, slot) dim: hs' = h*32 + s
MC_MLP = MLP_H // P    # 8
VW = 257               # vT tile width: 256 v-cols + 1 ones col
NCORES = 8
BP = B // NCORES       # 8 batch elems per core
NB_E = 2               # elements per resident group (slot loop batch)
NG = BP // NB_E        # 2 groups
SB = NB_E * S          # 96: batched slot free dim

# negative-seed rsqrt magic for half-scaled input (v05 = var/2), sign bit set
RSQRT_M = -1619374625  # int32 view of 0xDEF759DF


def _build_program(bp=BP):
    nc = bacc.Bacc(
        "TRN2",
        target_bir_lowering=False,
        debug=False,
        enable_asserts=False,
        num_devices=NCORES,
    )

    d = {}

    def din(name, shape, dt=f32):
        d[name] = nc.dram_tensor(name, shape, dt, kind="ExternalInput").ap()
        return d[name]

    xin = din("xin", [bp, KC, P, N], f32r)
    din("w1t", [P, KC, C], f32r)
    din("c1c", [P, KC])
    din("w2t", [P, KC, C], f32r)
    din("b2c", [P, KC])
    din("wkt", [P, KC, C], bf16)
    din("ckc", [P, KC])
    din("wvt", [P, KC, C], bf16)
    din("cvc", [P, KC])
    din("wqt", [P, KC, C], bf16)
    din("cqc", [P, KC])
    din("wit", [P, KC, 3 * SLOT], bf16)
    din("wht", [P, KC, 3 * SLOT], bf16)
    din("brzh", [P, 4])
    din("bhn", [P, KC])
    din("bin", [P, KC])
    din("m1t", [P, KC, MLP_H], bf16)
    din("c1m", [P, MC_MLP])
    din("m2t", [P, MC_MLP, C], bf16)
    din("b2m", [P, KC])
    din("wot", [P, KC, OUT], f32r)
    din("co", [1, OUT], f32r)
    din("smu", [P, KC, S])

    out_d = nc.dram_tensor("out", [bp, S, OUT], f32, kind="ExternalOutput").ap()

    from contextlib import ExitStack

    with tile.TileContext(nc) as tc, ExitStack() as ctx:
        wp = ctx.enter_context(tc.tile_pool(name="wp", bufs=1))
        kv = ctx.enter_context(tc.tile_pool(name="kv", bufs=2))
        ch = ctx.enter_context(tc.tile_pool(name="ch", bufs=2))
        cw = ctx.enter_context(tc.tile_pool(name="cw", bufs=2))
        xp = ctx.enter_context(tc.tile_pool(name="xp", bufs=2))
        t5 = ctx.enter_context(tc.tile_pool(name="t5", bufs=2))
        att = ctx.enter_context(tc.tile_pool(name="att", bufs=2))
        slo = ctx.enter_context(tc.tile_pool(name="slo", bufs=1))
        slp = ctx.enter_context(tc.tile_pool(name="slp", bufs=1))
        sl2 = ctx.enter_context(tc.tile_pool(name="sl2", bufs=2))
        ps = ctx.enter_context(tc.tile_pool(name="ps", bufs=2, space="PSUM"))

        def ptag(tag, shape, dt=f32):
            return ps.tile(shape, dt, tag=tag, bufs=2, name=tag)

        # ---- persistent constants / weights ----
        ident = wp.tile([P, P], f32, tag="ident")
        make_identity(nc, ident[:])
        ones_r = wp.tile([P, P], f32r, tag="ones_r")
        nc.vector.tensor_scalar(
            ones_r[:], ident[:], 0.0, 1.0, AluOpType.mult, AluOpType.add
        )
        ones_sb = wp.tile([1, SB], f32r, tag="ones_sb")
        nc.vector.tensor_scalar(
            ones_sb[:], ident[0:1, 0:SB], 0.0, 1.0, AluOpType.mult, AluOpType.add
        )
        ones_bf = wp.tile([P, P], bf16, tag="ones_bf")
        nc.vector.tensor_scalar(
            ones_bf[:], ident[:], 0.0, 1.0, AluOpType.mult, AluOpType.add
        )

        W = {}
        for name, ap in d.items():
            if name == "xin":
                continue
            t = wp.tile(list(ap.shape), ap.dtype, tag=name)
            nc.sync.dma_start(t[:], ap)
            W[name] = t

        # rstd = rsqrt(2*v05) via negative bit-trick seed + 1 Newton step.
        # No activation table needed (keeps ACT on the exp/tanh set all
        # kernel long). Destroys v05; sdt is scratch (holds -y0).
        def rsqrt2(dst, v05):
            # seed-only: dst = +bitcast(MAGICPOS - (bits(v05) >> 1)), ~3.4% err
            nc.vector.tensor_scalar(
                dst[:].bitcast(i32), v05[:].bitcast(i32), 1, None,
                AluOpType.logical_shift_right,
            )
            nc.vector.tensor_scalar(
                dst[:].bitcast(i32), dst[:].bitcast(i32), -1, RSQRT_M_POS,
                AluOpType.mult, AluOpType.add,
            )

        def rsqrt5(dst, v05, sdt, meng):
            nc.vector.tensor_scalar(
                sdt[:].bitcast(i32), v05[:].bitcast(i32), 1, None,
                AluOpType.logical_shift_right,
            )
            nc.vector.tensor_scalar(
                sdt[:].bitcast(i32), sdt[:].bitcast(i32), -1, RSQRT_M,
                AluOpType.mult, AluOpType.add,
            )
            meng.tensor_tensor(v05[:], v05[:], sdt[:], AluOpType.mult)
            meng.tensor_tensor(v05[:], v05[:], sdt[:], AluOpType.mult)
            nc.vector.scalar_tensor_tensor(
                dst[:], v05[:], 1.5, sdt[:], AluOpType.subtract, AluOpType.mult
            )

        # ---- phase A chunk emitter (one element, one 512-token chunk) ----
        def emit_chunk(e, nb, kb, vt):
            sl = ts(nb, NCH)
            x0 = ch.tile([P, KC, NCH], f32r, tag="x0c", name="x0")
            for kc in range(KC):
                nc.sync.dma_start(x0[:, kc], xin[e, kc, :, sl])
            # W1 (centered; LN_in scale cancels through the MLP)
            ph = [ptag("ph", [P, NCH]) for _ in range(KC)]
            for mc in range(KC):
                for kc in range(KC):
                    nc.tensor.matmul(
                        ph[mc][:], W["w1t"][:, kc, ts(mc, P)], x0[:, kc],
                        start=(kc == 0), stop=(kc == KC - 1),
                    )
            h = cw.tile([P, KC, NCH], f32r, tag="hc", name="h")
            for mc in range(KC):
                nc.scalar.activation(
                    h[:, mc], ph[mc][:], AF.Relu, bias=W["c1c"][:, mc : mc + 1]
                )
            # W2
            px2 = [ptag("px", [P, NCH]) for _ in range(KC)]
            for mc in range(KC):
                for kc in range(KC):
                    nc.tensor.matmul(
                        px2[mc][:], W["w2t"][:, kc, ts(mc, P)], h[:, kc],
                        start=(kc == 0), stop=(kc == KC - 1),
                    )
            x2 = cw.tile([P, KC, NCH], bf16, tag="x2c", name="x2")
            sq2 = cw.tile([P, KC, NCH], bf16, tag="sq2", name="sq2")
            for mc in range(KC):
                nc.scalar.activation(
                    x2[:, mc], px2[mc][:], AF.Identity, bias=W["b2c"][:, mc : mc + 1]
                )
                nc.scalar.activation(
                    sq2[:, mc], px2[mc][:], AF.Square, bias=W["b2c"][:, mc : mc + 1]
                )
            # LN_inp stats: mean and E[x^2] via ones-matmul
            p1 = ptag("pkv", [P, NCH])
            p2 = ptag("pkv", [P, NCH])
            for kc in range(KC):
                nc.tensor.matmul(
                    p1[:], ones_bf[:], x2[:, kc],
                    start=(kc == 0), stop=(kc == KC - 1),
                )
            for kc in range(KC):
                nc.tensor.matmul(
                    p2[:], ones_bf[:], sq2[:, kc],
                    start=(kc == 0), stop=(kc == KC - 1),
                )
            CSEED = (1.0 + 0.017) ** 2
            sqm = t5.tile([P, NCH], f32, tag="sqm", name="sqm")
            nc.scalar.activation(
                sqm[:], p1[:], AF.Square, scale=(0.5 * CSEED) ** 0.5 / C
            )
            v05 = t5.tile([P, NCH], f32, tag="v05", bufs=1, name="v05")
            nc.vector.scalar_tensor_tensor(
                v05[:], p2[:], 0.5 * CSEED / C, sqm[:],
                AluOpType.mult, AluOpType.subtract,
            )
            ivb = t5.tile([P, NCH], f32, tag="sqm", name="ivb")
            rsqrt2(ivb, v05)
            xh2 = xp.tile([P, KC, NCH], bf16, tag="xh2", name="xh2")
            nc.vector.tensor_tensor(
                xh2[:], x2[:], ivb[:, None, :].broadcast_to([P, KC, NCH]),
                AluOpType.mult,
            )
            # K projection: slot-major output [d, tok]
            pk = [ptag("pkv", [P, NCH]) for _ in range(KC)]
            for mc in range(KC):
                for kc in range(KC):
                    nc.tensor.matmul(
                        pk[mc][:], W["wkt"][:, kc, ts(mc, P)], xh2[:, kc],
                        start=(kc == 0), stop=(kc == KC - 1),
                    )
            nc.scalar.activation(
                kb[:, 0, sl], pk[0][:], AF.Identity, bias=W["ckc"][:, 0:1]
            )
            nc.vector.tensor_scalar_add(kb[:, 1, sl], pk[1][:], W["ckc"][:, 1:2])
            # V projection: token-major output [tok, d] (x stationary)
            for jp in range(2):
                pv = ptag("pkv", [P, 2, C])
                for jj in range(2):
                    j = 2 * jp + jj
                    for kc in range(KC):
                        nc.tensor.matmul(
                            pv[:, jj], xh2[:, kc, ts(j, P)], W["wvt"][:, kc, :],
                            start=(kc == 0), stop=(kc == KC - 1),
                        )
                nc.gpsimd.tensor_copy(
                    vt[:, nb * 4 + 2 * jp : nb * 4 + 2 * jp + 2, 0:256], pv[:]
                )

        def slot_stats(src_f32, tag):
            """src [P, KC, SB] -> iv [P, SB] (rstd via DVE-only rsqrt)."""
            sqs = slo.tile([P, KC, SB], f32r, tag="ssq", name="sqs")
            nc.gpsimd.tensor_mul(sqs[:], src_f32[:], src_f32[:])
            pq1 = ptag("lng", [P, SB])
            for kc in range(KC):
                nc.tensor.matmul(
                    pq1[:], ones_r[:], src_f32[:, kc],
                    start=(kc == 0), stop=(kc == KC - 1),
                )
            pq2 = ptag("lng", [P, SB])
            for kc in range(KC):
                nc.tensor.matmul(
                    pq2[:], ones_r[:], sqs[:, kc],
                    start=(kc == 0), stop=(kc == KC - 1),
                )
            sqm = slo.tile([P, SB], f32, tag="ssqm", name="sqm2")
            nc.scalar.activation(sqm[:], pq1[:], AF.Square, scale=0.5 ** 0.5 / C)
            vv = slo.tile([P, SB], f32, tag="sv", name="vv")
            nc.vector.scalar_tensor_tensor(
                vv[:], pq2[:], 0.5 / C, sqm[:], AluOpType.mult, AluOpType.subtract
            )
            sdt = slo.tile([P, SB], f32, tag="rssd", name="sdt2")
            iv = slo.tile([P, SB], f32, tag=tag + "iv", name="iv")
            rsqrt5(iv, vv, sdt, nc.vector)
            return iv

        # ---- slot-loop piece list for one group (emitted lazily) ----
        def make_slot_pieces(g, kbf, vtt):
            st = {}

            def p_init():
                slots = sl2.tile([P, KC, SB], f32r, tag="slots", name="slots")
                for e4 in range(NB_E):
                    nc.vector.tensor_copy(
                        slots[:, :, e4 * S : (e4 + 1) * S], W["smu"][:]
                    )
                st["slots"] = slots
                qb = []
                for e4 in range(NB_E):
                    q = slp.tile([P, KC, HSP], bf16, tag=f"qb{e4}", name="q")
                    nc.vector.memset(q[:], 0.0)
                    qb.append(q)
                st["qb"] = qb

            def p_stats(it):
                slots, qb = st["slots"], st["qb"]
                ivq = slot_stats(slots, "qs")
                sh = slo.tile([P, KC, SB], bf16, tag="sh", name="sh")
                for kc in range(KC):
                    nc.gpsimd.tensor_mul(sh[:, kc], slots[:, kc], ivq[:])
                qsb = slo.tile([P, KC, SB], bf16, tag="qsb", name="qsb")
                for mc in range(KC):
                    pq = ptag("lng", [P, SB])
                    for kc in range(KC):
                        nc.tensor.matmul(
                            pq[:], W["wqt"][:, kc, ts(mc, P)], sh[:, kc],
                            start=(kc == 0), stop=(kc == KC - 1),
                        )
                    nc.scalar.activation(
                        qsb[:, mc], pq[:], AF.Identity, bias=W["cqc"][:, mc : mc + 1]
                    )
                for e4 in range(NB_E):
                    for hh in range(H):
                        pr = slice((hh % 2) * 64, (hh % 2) * 64 + 64)
                        nc.gpsimd.tensor_copy(
                            qb[e4][pr, hh // 2, hh * 32 : hh * 32 + S],
                            qsb[pr, hh // 2, e4 * S : e4 * S + S],
                        )
                st["updt"] = slp.tile([P, KC, SB], bf16, tag="updt", name="updt")

            def p_att(it, e4):
                qb = st["qb"]
                updt = st["updt"]
                psu = ptag("lng", [P, SLOT + 1])
                for gi in range(8):
                    psl = ptag("pkv", [P, 4, HSP])
                    for j4 in range(4):
                        nl = gi * 4 + j4
                        # heads {0,1} live in k-chunk 0 / hs 0:64; {2,3} in
                        # chunk 1 / hs 64:128 (qb is block-sparse) — two
                        # independent half-width matmuls, no accumulation.
                        for kc in range(KC):
                            nc.tensor.matmul(
                                psl[:, j4, kc * 64 : kc * 64 + 64],
                                kbf[e4][:, kc, ts(nl, P)],
                                qb[e4][:, kc, kc * 64 : kc * 64 + 64],
                                start=True, stop=True,
                            )
                    esb = att.tile([P, 4, HSP], bf16, tag="esb", bufs=5, name="esb")
                    nc.scalar.activation(esb[:], psl[:], AF.Exp)
                    t4 = att.tile([P, 4], f32, tag="t4", bufs=4, name="t4")
                    nc.vector.reduce_sum(t4[:], esb[:], axis=AX.X)
                    t4m = att.tile([P, 4], f32, tag="t4m", bufs=4, name="t4m")
                    nc.vector.tensor_scalar(
                        t4m[:], t4[:], -32.0, None, AluOpType.add
                    )
                    rt4 = att.tile([P, 4], f32, tag="rt4", bufs=4, name="rt4")
                    nc.vector.reciprocal_approx_fast(rt4[:], t4m[:])
                    nc.vector.tensor_tensor(
                        esb[:], esb[:],
                        rt4[:, :, None].broadcast_to([P, 4, HSP]),
                        AluOpType.mult,
                    )
                    for j4 in range(4):
                        nc.tensor.matmul(
                            psu[:], esb[:, j4], vtt[e4][:, gi * 4 + j4, :],
                            start=(gi == 0 and j4 == 0),
                            stop=(gi == 7 and j4 == 3),
                            skip_group_check=True,
                        )
                rz = att.tile([P, 1], f32, tag="rz", name="rz")
                nc.vector.reciprocal_approx_fast(rz[:], psu[:, 256:257])
                upd_s = att.tile([P, SLOT], f32, tag="upd_s", name="upd_s")
                nc.vector.tensor_scalar_mul(upd_s[:], psu[:, 0:SLOT], rz[:])
                for hh in range(H):
                    bp0 = hh * 32
                    pt = ptag("lng", [64, S])
                    nc.tensor.transpose(
                        pt[:],
                        upd_s[bp0 : bp0 + S, ts(hh, DH)],
                        ident[bp0 : bp0 + S, bp0 : bp0 + S],
                        tile_position=(bp0, 0),
                    )
                    nc.scalar.activation(
                        updt[(hh % 2) * 64 : (hh % 2) * 64 + 64, hh // 2,
                             e4 * S : e4 * S + S],
                        pt[:],
                        AF.Identity,
                        bias=W["cvc"][(hh % 2) * 64 : (hh % 2) * 64 + 64,
                                      hh // 2 : hh // 2 + 1],
                    )

            def p_gru(it):
                slots, updt = st["slots"], st["updt"]
                sl16 = slo.tile([P, KC, SB], bf16, tag="sl16", name="sl16")
                nc.gpsimd.tensor_copy(sl16[:], slots[:])
                g12 = ptag("lng", [P, 8, SB])
                ph_rz = g12[:, 0:4]
                px_rz = g12[:, 4:8]
                pn = ptag("lng", [P, 4, SB])
                for gj in range(4):
                    for kc in range(KC):
                        nc.tensor.matmul(
                            ph_rz[:, gj], W["wht"][:, kc, ts(gj, P)], sl16[:, kc],
                            start=(kc == 0), stop=(kc == KC - 1),
                        )
                for gj in range(4):
                    for kc in range(KC):
                        nc.tensor.matmul(
                            px_rz[:, gj], W["wit"][:, kc, ts(gj, P)], updt[:, kc],
                            start=(kc == 0), stop=(kc == KC - 1),
                        )
                for nj in range(KC):
                    for kc in range(KC):
                        nc.tensor.matmul(
                            pn[:, nj], W["wit"][:, kc, ts(4 + nj, P)], updt[:, kc],
                            start=(kc == 0), stop=(kc == KC - 1),
                        )
                    for kc in range(KC):
                        nc.tensor.matmul(
                            pn[:, 2 + nj], W["wht"][:, kc, ts(4 + nj, P)],
                            sl16[:, kc],
                            start=(kc == 0), stop=(kc == KC - 1),
                        )
                hgs = slo.tile([P, 4, SB], bf16, tag="trz", name="hgs")
                nc.scalar.activation(hgs[:], ph_rz, AF.Identity)
                tg = slo.tile([P, 4, SB], f32, tag="tg", name="tg")
                nc.vector.tensor_add(tg[:], px_rz, hgs[:])
                trz = slo.tile([P, 4, SB], f32, tag="trz", name="trz")
                for gj in range(4):
                    nc.scalar.activation(
                        trz[:, gj], tg[:, gj], AF.Tanh, scale=0.5,
                        bias=W["brzh"][:, gj : gj + 1],
                    )
                # n = tanh(0.5*(y + tr*y) + xn + bin), y = hn + bhn
                pns = slo.tile([P, 4, SB], f32, tag="tg", name="pns")
                nc.scalar.activation(pns[:], pn[:], AF.Identity)
                yn = slo.tile([P, KC, SB], f32, tag="yn", name="yn")
                for nj in range(KC):
                    nc.vector.tensor_scalar_add(
                        yn[:, nj], pns[:, 2 + nj], W["bhn"][:, nj : nj + 1]
                    )
                gn = slo.tile([P, KC, SB], f32, tag="gn", name="gn")
                nc.vector.tensor_mul(gn[:], trz[:, 0:2], yn[:])
                nc.vector.tensor_add(gn[:], gn[:], yn[:])
                mn = slo.tile([P, KC, SB], f32, tag="ssq", name="mn")
                for nj in range(KC):
                    nc.vector.scalar_tensor_tensor(
                        mn[:, nj], gn[:, nj], 0.5, pns[:, nj],
                        AluOpType.mult, AluOpType.add,
                    )
                nsb = slo.tile([P, KC, SB], f32, tag="nsb", name="nsb")
                for nj in range(KC):
                    nc.scalar.activation(
                        nsb[:, nj], mn[:, nj], AF.Tanh, bias=W["bin"][:, nj : nj + 1]
                    )
                # slots2 = n + (0.5 + 0.5*tz)*(slots - n)
                dd = slo.tile([P, KC, SB], f32, tag="dd", name="dd")
                nc.vector.tensor_sub(dd[:], slots[:], nsb[:])
                ee = slo.tile([P, KC, SB], f32, tag="ee", name="ee")
                nc.vector.tensor_mul(ee[:], trz[:, 2:4], dd[:])
                nc.vector.tensor_add(ee[:], ee[:], dd[:])
                slots2 = slp.tile([P, KC, SB], f32r, tag="slots2", name="slots2")
                nc.vector.scalar_tensor_tensor(
                    slots2[:], ee[:], 0.5, nsb[:], AluOpType.mult, AluOpType.add
                )
                st["slots2"] = slots2

            def p_mlp(it):
                slots2 = st["slots2"]
                ivm = slot_stats(slots2, "ms")
                sh2 = slo.tile([P, KC, SB], bf16, tag="sh2", name="sh2")
                for kc in range(KC):
                    nc.gpsimd.tensor_mul(sh2[:, kc], slots2[:, kc], ivm[:])
                hm = slo.tile([P, MC_MLP, SB], bf16, tag="hm", name="hm")
                for j in range(MC_MLP):
                    pz = ptag("lng", [P, SB])
                    for kc in range(KC):
                        nc.tensor.matmul(
                            pz[:], W["m1t"][:, kc, ts(j, P)], sh2[:, kc],
                            start=(kc == 0), stop=(kc == KC - 1),
                        )
                    if j % 2 == 0:
                        nc.vector.tensor_scalar(
                            hm[:, j], pz[:], W["c1m"][:, j : j + 1], 0.0,
                            AluOpType.add, AluOpType.max,
                        )
                    else:
                        nc.scalar.activation(
                            hm[:, j], pz[:], AF.Relu, bias=W["c1m"][:, j : j + 1]
                        )
                slots3 = sl2.tile([P, KC, SB], f32r, tag="slots", name="slots3")
                for mc in range(KC):
                    p2z = ptag("lng", [P, SB])
                    for j in range(MC_MLP):
                        nc.tensor.matmul(
                            p2z[:], W["m2t"][:, j, ts(mc, P)], hm[:, j],
                            start=(j == 0), stop=(j == MC_MLP - 1),
                        )
                    tr = slo.tile([P, SB], f32, tag="tr", name="tr")
                    nc.vector.tensor_scalar_add(
                        tr[:], p2z[:], W["b2m"][:, mc : mc + 1]
                    )
                    nc.vector.tensor_add(slots3[:, mc], tr[:], slots2[:, mc])
                st["slots"] = slots3

            def p_head():
                slots = st["slots"]
                ivh = slot_stats(slots, "hs")
                sh3 = slo.tile([P, KC, SB], f32r, tag="sh3", name="sh3")
                for kc in range(KC):
                    nc.gpsimd.tensor_mul(sh3[:, kc], slots[:, kc], ivh[:])
                po = ptag("lng", [SB, OUT])
                for kc in range(KC):
                    nc.tensor.matmul(
                        po[:], sh3[:, kc], W["wot"][:, kc, :],
                        start=(kc == 0), stop=False,
                    )
                nc.tensor.matmul(
                    po[:], ones_sb[:], W["co"][:], start=False, stop=True
                )
                osb = slo.tile([SB, OUT], f32, tag="osb", name="osb")
                nc.scalar.activation(osb[:], po[:], AF.Copy)
                for e4 in range(NB_E):
                    nc.sync.dma_start(
                        out_d[g * NB_E + e4], osb[e4 * S : (e4 + 1) * S, :]
                    )

            pieces = [p_init]
            for it in range(ITERS):
                pieces.append(lambda it=it: p_stats(it))
                for e4 in range(NB_E):
                    pieces.append(lambda it=it, e4=e4: p_att(it, e4))
                pieces.append(lambda it=it: p_gru(it))
                pieces.append(lambda it=it: p_mlp(it))
            pieces.append(p_head)
            return pieces

        # ======== pipelined emission: phase A(g) overlaps slot loop(g-1) =====
        pend = []
        for g in range(bp // NB_E):
            kbf = []
            vtt = []
            for e4 in range(NB_E):
                kb = kv.tile([P, KC, N], bf16, tag=f"kbf{e4}", name="kb")
                vt = kv.tile([P, NL, VW], bf16, tag=f"vtt{e4}", name="vt")
                nc.vector.memset(vt[:, :, 256:257], 1.0)
                kbf.append(kb)
                vtt.append(vt)
            chunks = [(nb, e4) for nb in range(NB) for e4 in range(NB_E)]
            npcs = len(pend)
            pi = 0
            for i, (nb, e4) in enumerate(chunks):
                emit_chunk(g * NB_E + e4, nb, kbf[e4], vtt[e4])
                while pi < npcs and (pi - 1) * len(chunks) < (i + 1) * npcs:
                    pend[pi]()
                    pi += 1
            while pi < npcs:
                pend[pi]()
                pi += 1
            pend = make_slot_pieces(g, kbf, vtt)
        for p in pend:
            p()

    nc.compile()
    return nc


def _center(a):
    """Column-center: x @ center(A) == (x - mean(x)) @ A."""
    return (a - a.mean(0, keepdims=True)).astype(np.float32)


def _host_prepack(i):
    g = lambda k: np.asarray(i[k], np.float32)
    coords = (np.arange(RES, dtype=np.float32) + 0.5) / RES
    gx = np.broadcast_to(coords[None, :], (RES, RES))
    gy = np.broadcast_to(coords[:, None], (RES, RES))
    pe = np.stack([gx, gy, 1.0 - gx, 1.0 - gy], 0).astype(np.float32)
    pos = np.einsum("co,chw->ohw", g("pos_w"), pe).astype(np.float32)
    pos = pos + g("pos_b")[:, None, None]
    x = g("inputs") + pos[None]
    xin = np.ascontiguousarray(x.reshape(B, KC, P, N))

    def kmaj(w, dt=np.float32):
        K, M = w.shape
        return np.ascontiguousarray(
            w.reshape(K // P, P, M).transpose(1, 0, 2).astype(dt)
        )

    def cols(v):
        M = v.shape[0]
        return np.ascontiguousarray(v.reshape(M // P, P).T.astype(np.float32))

    sh = {}
    # W1: gamma-fold + center (mean of LN_in via centering; scale cancels)
    sh["w1t"] = kmaj(_center(g("ln_in_g")[:, None] * g("mlp_in_w1")))
    sh["c1c"] = cols(g("ln_in_b") @ g("mlp_in_w1") + g("mlp_in_b1"))
    sh["w2t"] = kmaj(g("mlp_in_w2"))
    sh["b2c"] = cols(g("mlp_in_b2"))
    ks = float(SLOT) ** -0.5
    sh["wkt"] = kmaj(_center(g("ln_inp_g")[:, None] * g("Wk")) * ks, BF)
    sh["ckc"] = cols((g("ln_inp_b") @ g("Wk")) * ks)
    sh["wvt"] = kmaj(_center(g("ln_inp_g")[:, None] * g("Wv")), BF)
    sh["cvc"] = cols(g("ln_inp_b") @ g("Wv"))
    sh["wqt"] = kmaj(_center(g("ln_slot_g")[:, None] * g("Wq")), BF)
    sh["cqc"] = cols(g("ln_slot_b") @ g("Wq"))
    sh["wit"] = kmaj(g("gru_wi"), BF)
    sh["wht"] = kmaj(g("gru_wh"), BF)
    bsum = g("gru_bi") + g("gru_bh")
    sh["brzh"] = cols(0.5 * bsum[0 : 2 * SLOT])
    sh["bhn"] = cols(g("gru_bh")[2 * SLOT :])
    sh["bin"] = cols(g("gru_bi")[2 * SLOT :])
    sh["m1t"] = kmaj(_center(g("ln_mlp_g")[:, None] * g("mlp_w1")), BF)
    sh["c1m"] = cols(g("ln_mlp_b") @ g("mlp_w1") + g("mlp_b1"))
    sh["m2t"] = kmaj(g("mlp_w2"), BF)
    sh["b2m"] = cols(g("mlp_b2"))
    sh["wot"] = kmaj(_center(g("ln_out_g")[:, None] * g("out_w")))
    sh["co"] = (g("ln_out_b") @ g("out_w") + g("out_b")).reshape(1, OUT)
    mu = np.asarray(i["slot_mu"], np.float32)[0]
    sh["smu"] = np.ascontiguousarray(mu.T.reshape(KC, P, S).transpose(1, 0, 2))
    return sh, xin


_NC_CACHE = {}
LAST_RESULTS = None


def _get_nc():
    if "nc" not in _NC_CACHE:
        _NC_CACHE["nc"] = _build_program(BP)
    return _NC_CACHE["nc"]


def kernel(**inputs):
    global LAST_RESULTS
    nc = _get_nc()
    sh, xin = _host_prepack(inputs)
    in_maps = []
    for c in range(NCORES):
        m = dict(sh)
        m["xin"] = np.ascontiguousarray(xin[c * BP : (c + 1) * BP])
        in_maps.append(m)
    res = bass_utils.run_bass_kernel_spmd(
        nc, in_maps, core_ids=list(range(NCORES))
    )
    LAST_RESULTS = res
    out = np.concatenate([res.results[c]["out"] for c in range(NCORES)], 0)
    return out.astype(np.float32)

